# revision 35
# baseline (speedup 1.0000x reference)
import math

import numpy as np

# nn_DescLayer: LayerNorm -> x@M^T, x@R^T -> Nk[b,s,i] = sum_{j,g} P[i,j,g] *
# cos(2*pi*k[b,s]/periods[i,j,g]) * xproj[b,s,j]; out = res + Nk.
# Data-parallel over the 1024 (b,s) tokens: 128 tokens per NeuronCore.
#
# Per-core layout: periods flattened as flat = i*512 + j*8 + g are viewed as
# (p, i, glo) with p = j*2 + (g>>2) on partitions and (i, glo=g&3) in the
# free dim (flat = i*512 + p*4 + glo). Then xproj[t, j] is constant per
# partition (j = p>>1), so one fused scalar_tensor_tensor computes
# (sin ⊙ xp) ⊙ P per token, and the (j,g)-contraction is a plain
# partition-sum on the TensorEngine: stationary = ones(128,1) (loaded once),
# moving = fp16 product tiles batched 8 tokens x 64 i x 4 glo-accumulated
# matmuls into PSUM rows that are already in flat y order.
#
# cos(2pi*k/p) = sin(2pi*k/p + pi/2) is valid for the Sin LUT ([-pi, pi])
# whenever k/p <= 1/4, i.e. for all periods with i >= 4. The first 2048
# periods (i < 4) go through a token-major pass with explicit
# round-to-nearest range reduction: f = (k/p + 1/4) - round(k/p + 1/4),
# cos = sin(2pi*f).

B, S, D, NB = 2, 512, 64, 8
N_CORES = 8
TOK = (B * S) // N_CORES  # 128 tokens per core
NPER = D * D * NB  # 32768 periods
SMALL = 2048  # flat period idx < SMALL (i<4) needs range reduction
LN_EPS = 1e-5
TWO_PI = 2.0 * math.pi
RND_C = 12582912.0  # 1.5 * 2**23: (u + C) - C == round-to-nearest(u) in f32

GRP = 8  # tokens per PE row-matmul group
PATH_B = 0  # tokens per group whose sin angle is computed on VectorE
USE_STT = True  # fused (sin*xp)*P in one scalar_tensor_tensor
MAIN_16 = True  # fp16 for sin/P/prod tiles

_CACHE = {}


def _split_waits(nc, maxw=1):
    """This walrus build rejects instructions carrying more than one sem
    wait. Hoist excess waits onto same-engine NoOps placed immediately
    before the instruction (same engine stream => executes first)."""
    import bass_rust
    import concourse.mybir as mybir

    ctr = [0]
    for f in nc.m.functions:
        for b in f.blocks:
            new_insts = []
            changed = False
            for inst in b.instructions:
                si = inst.sync_info
                waits = list(si.on_wait) if si and si.on_wait else []
                if len(waits) > maxw:
                    keep = waits[-maxw:]
                    extra = waits[:-maxw]
                    for i0 in range(0, len(extra), maxw):
                        ctr[0] += 1
                        nop = bass_rust.InstNoOp(
                            name=f"I-waitsplit-{ctr[0]}",
                            engine=inst.engine,
                            text_hint="waitsplit",
                            sync_info=mybir.SyncInfo(
                                on_wait=extra[i0 : i0 + maxw], on_update=[]
                            ),
                        )
                        new_insts.append(nop)
                    si.on_wait = keep
                    changed = True
                new_insts.append(inst)
            if changed:
                b.instructions = new_insts


def _build_program(split=True):
    import concourse.bass as bass
    import concourse.mybir as mybir
    from concourse.tile import TileContext
    from concourse.vector_clock import ScopedClock, VectorClock

    # --- workaround: walrus rejects >1 sem wait on the Tile tail drain;
    # spread the waits over SP nops (1 each), then issue a bare drain.
    def _drain_and_barrier(self, tick_clock, wait_clock):
        nc = self.nc
        gc = tick_clock.global_clock
        n = len(gc)
        for i in range(n):
            tick = gc[i]
            if tick <= 0:
                continue
            vec = [0] * n
            vec[i] = tick
            nop_inst = nc.sync.nop(nofuse=True, hint=f"drain_wait_{i}")
            wait_clock.add_sem_waits(
                nop_inst.ins, ScopedClock({None: VectorClock(vec)})
            )
        nc.sync.drain()
        nc.all_engine_barrier()
        assert self.sems is not None
        popped = nc._tile_sem_poison_stack.pop()
        assert popped is self._sem_poison
        nc.clear_and_free_semaphores(list(self.sems.allocated().values()))
        nc.all_engine_barrier()

    TileContext._drain_and_barrier = _drain_and_barrier

    f32 = mybir.dt.float32
    f16 = mybir.dt.float16 if MAIN_16 else mybir.dt.float32
    i32 = mybir.dt.int32
    AF = mybir.ActivationFunctionType
    OP = mybir.AluOpType
    AX = mybir.AxisListType

    nc = bass.Bass()
    X = nc.declare_dram_parameter("x", [TOK, D], f32, isOutput=False)
    K = nc.declare_dram_parameter("k", [1, TOK], f32, isOutput=False)
    Mw = nc.declare_dram_parameter("M", [D, D], f32, isOutput=False)
    Rw = nc.declare_dram_parameter("R", [D, D], f32, isOutput=False)
    Pw = nc.declare_dram_parameter("P", [NPER], f32, isOutput=False)
    Gam = nc.declare_dram_parameter("gamma", [1, D], f32, isOutput=False)
    Bet = nc.declare_dram_parameter("beta", [1, D], f32, isOutput=False)
    Per = nc.declare_dram_parameter("periods", [NPER], f32, isOutput=False)
    Ones = nc.declare_dram_parameter("ones", [1, 128], f32, isOutput=False)
    Rep = nc.declare_dram_parameter("rep", [D, 128], f32, isOutput=False)
    Idm = nc.declare_dram_parameter("idm", [128, 128], f32, isOutput=False)
    Y = nc.declare_dram_parameter("y", [TOK, D], f32, isOutput=True)

    NGRP = TOK // GRP

    with TileContext(nc) as tc:
        with (
            tc.tile_pool(name="const", bufs=1) as cp,
            tc.tile_pool(name="ang", bufs=3) as angp,
            tc.tile_pool(name="sino", bufs=3) as sinp,
            tc.tile_pool(name="prod", bufs=3) as prodp,
            tc.tile_pool(name="prow", bufs=4, space="PSUM") as prowp,
            tc.tile_pool(name="pprep", bufs=3, space="PSUM") as pprep,
            tc.tile_pool(name="dram", bufs=1, space="DRAM") as dramp,
        ):
            scratch = dramp.tile([TOK * D], f32, tag="scr")
            # ---------------- load constants ----------------
            xs = cp.tile([TOK, D], f32, tag="xs")
            nc.sync.dma_start(out=xs[:], in_=X[:])
            kr = cp.tile([1, TOK], f32, tag="kr")
            nc.sync.dma_start(out=kr[:], in_=K[:])
            mn = cp.tile([D, D], f32, tag="mn")
            nc.sync.dma_start(out=mn[:], in_=Mw[:])
            rn = cp.tile([D, D], f32, tag="rn")
            nc.sync.dma_start(out=rn[:], in_=Rw[:])
            gam = cp.tile([1, D], f32, tag="gam")
            nc.sync.dma_start(out=gam[:], in_=Gam[:])
            bet = cp.tile([1, D], f32, tag="bet")
            nc.sync.dma_start(out=bet[:], in_=Bet[:])
            onesr = cp.tile([1, 128], f32, tag="onesr")
            nc.sync.dma_start(out=onesr[:], in_=Ones[:])
            rep = cp.tile([D, 128], f32, tag="rep")
            nc.sync.dma_start(out=rep[:], in_=Rep[:])
            idm = cp.tile([128, 128], f32, tag="idm")
            nc.sync.dma_start(out=idm[:], in_=Idm[:])

            # periods/P in (p, i, glo) layout: flat = i*512 + p*4 + glo
            pert2 = cp.tile([128, 256], f32, tag="pert2")
            nc.sync.dma_start(
                out=pert2[:].rearrange("p (i glo) -> p i glo", glo=4),
                in_=Per[:].rearrange("(i p glo) -> p i glo", i=D, p=128, glo=4),
            )
            pt2 = cp.tile([128, 256], f32, tag="pt2")
            nc.sync.dma_start(
                out=pt2[:].rearrange("p (i glo) -> p i glo", glo=4),
                in_=Pw[:].rearrange("(i p glo) -> p i glo", i=D, p=128, glo=4),
            )
            invp2 = cp.tile([128, 256], f32, tag="invp2")
            nc.vector.reciprocal(invp2[:], pert2[:])
            # copy with i<4 zeroed for the main-path activations: keeps every
            # sin argument inside the LUT range [-pi, pi] (those columns'
            # products are zeroed via p2h anyway; sin(pi/2)=1 is harmless)
            invp2c = cp.tile([128, 256], f32, tag="invp2c")
            nc.vector.tensor_copy(invp2c[:], invp2[:])
            nc.vector.tensor_scalar(
                invp2c[:, 0:16], invp2c[:, 0:16], 0.0, None, OP.mult
            )
            invp2g = cp.tile([128, 256], f32, tag="invp2g")
            nc.vector.tensor_copy(
                invp2g[:].rearrange("p (glo i) -> p i glo", glo=4),
                invp2c[:].rearrange("p (i glo) -> p i glo", glo=4),
            )

            # P as fp16 in (glo, i) order with i<4 zeroed (small-p pass owns
            # those; zeroing also kills the dummy sin values there). The
            # (glo, i) order makes each glo slice of the product tile a
            # contiguous, 4B-aligned matmul moving operand.
            p2hg = cp.tile([128, 256], f16, tag="p2hg")
            nc.vector.tensor_copy(
                p2hg[:].rearrange("p (glo i) -> p i glo", glo=4),
                pt2[:].rearrange("p (i glo) -> p i glo", glo=4),
            )
            p2hg3 = p2hg[:].rearrange("p (glo i) -> p glo i", glo=4)
            nc.vector.tensor_scalar(
                p2hg3[:, :, 0:4], p2hg3[:, :, 0:4], 0.0, None, OP.mult
            )

            p0r = cp.tile([1, SMALL], f32, tag="p0r")
            nc.sync.dma_start(
                out=p0r[:], in_=Pw[0:SMALL].rearrange("(a b) -> a b", a=1)
            )

            # activation per-partition bias vectors
            bias_hp = cp.tile([128, 1], f32, tag="bias_hp")
            nc.vector.memset(bias_hp[:], math.pi / 2.0)
            bias_z = cp.tile([128, 1], f32, tag="bias_z")
            nc.vector.memset(bias_z[:], 0.0)
            ones_h = cp.tile([128, 1], f16, tag="ones_h")
            nc.vector.memset(ones_h[:], 1.0)

            # ---------------- PE prep ----------------
            id64 = idm[0:64, 0:64]

            mt_ps = pprep.tile([D, D], f32, tag="pp")
            nc.tensor.transpose(mt_ps[:], mn[:], id64)
            mt = cp.tile([D, D], f32, tag="mt")
            nc.vector.tensor_copy(mt[:], mt_ps[:])

            rt_ps = pprep.tile([D, D], f32, tag="pp")
            nc.tensor.transpose(rt_ps[:], rn[:], id64)
            rt = cp.tile([D, D], f32, tag="rt")
            nc.vector.tensor_copy(rt[:], rt_ps[:])

            gb_ps = pprep.tile([128, D], f32, tag="pp")
            nc.tensor.matmul(gb_ps[:], onesr[:], gam[:], start=True, stop=True)
            gb = cp.tile([128, D], f32, tag="gb")
            nc.vector.tensor_copy(gb[:], gb_ps[:])
            bb_ps = pprep.tile([128, D], f32, tag="pp")
            nc.tensor.matmul(bb_ps[:], onesr[:], bet[:], start=True, stop=True)
            bb = cp.tile([128, D], f32, tag="bb")
            nc.vector.tensor_copy(bb[:], bb_ps[:])

            kv_ps = pprep.tile([TOK, 1], f32, tag="pp")
            nc.tensor.matmul(kv_ps[:], kr[:], onesr[:, 0:1], start=True, stop=True)
            kvec = cp.tile([TOK, 1], f32, tag="kvec")
            nc.vector.tensor_copy(kvec[:], kv_ps[:])

            k2_ps = pprep.tile([128, TOK], f32, tag="pp")
            nc.tensor.matmul(k2_ps[:], onesr[:], kr[:], start=True, stop=True)
            k2pi = cp.tile([128, TOK], f32, tag="k2pi")
            nc.vector.tensor_scalar(k2pi[:], k2_ps[:], TWO_PI, None, OP.mult)

            # ---------------- LayerNorm (token-major) ----------------
            rsum = cp.tile([TOK, 1], f32, tag="rsum")
            nc.vector.tensor_reduce(rsum[:], xs[:], AX.X, OP.add)
            mu = cp.tile([TOK, 1], f32, tag="mu")
            nc.vector.tensor_scalar(mu[:], rsum[:], 1.0 / D, None, OP.mult)
            cen = cp.tile([TOK, D], f32, tag="cen")
            nc.vector.tensor_scalar(cen[:], xs[:], mu[:], None, OP.subtract)
            sq = cp.tile([TOK, D], f32, tag="sq")
            nc.vector.tensor_tensor(sq[:], cen[:], cen[:], OP.mult)
            ssq = cp.tile([TOK, 1], f32, tag="ssq")
            nc.vector.tensor_reduce(ssq[:], sq[:], AX.X, OP.add)
            veps = cp.tile([TOK, 1], f32, tag="veps")
            nc.vector.tensor_scalar(veps[:], ssq[:], 1.0 / D, LN_EPS, OP.mult, OP.add)

            # rstd = 1/sqrt(veps): bit-hack seed + 3 Newton steps (keeps
            # ScalarE's activation table on the trig set only)
            ti = cp.tile([TOK, 1], i32, tag="ti")
            nc.vector.tensor_scalar(
                ti[:], veps[:].bitcast(i32), 1, -1, OP.arith_shift_right,
                OP.bitwise_xor,
            )
            yr = cp.tile([TOK, 1], f32, tag="yr")
            nc.vector.tensor_scalar(
                yr[:].bitcast(i32), ti[:], 0x5F3759DF + 1, None, OP.add
            )
            hh = cp.tile([TOK, 1], f32, tag="hh")
            nc.vector.tensor_scalar(hh[:], veps[:], 0.5, None, OP.mult)
            for it in range(3):
                t1 = cp.tile([TOK, 1], f32, tag=f"nt1_{it}")
                nc.vector.tensor_tensor(t1[:], yr[:], yr[:], OP.mult)
                t2 = cp.tile([TOK, 1], f32, tag=f"nt2_{it}")
                nc.vector.tensor_tensor(t2[:], t1[:], hh[:], OP.mult)
                t3 = cp.tile([TOK, 1], f32, tag=f"nt3_{it}")
                nc.vector.tensor_scalar(t3[:], t2[:], 1.5, -1.0, OP.subtract, OP.mult)
                yn = cp.tile([TOK, 1], f32, tag=f"nt4_{it}")
                nc.vector.tensor_tensor(yn[:], yr[:], t3[:], OP.mult)
                yr = yn

            ln0 = cp.tile([TOK, D], f32, tag="ln0")
            nc.vector.tensor_scalar(ln0[:], cen[:], yr[:], None, OP.mult)
            ln1 = cp.tile([TOK, D], f32, tag="ln1")
            nc.vector.tensor_tensor(ln1[:], ln0[:], gb[:], OP.mult)
            lnf = cp.tile([TOK, D], f32, tag="lnf")
            nc.vector.tensor_tensor(lnf[:], ln1[:], bb[:], OP.add)

            # ---------------- projections ----------------
            lnT_ps = pprep.tile([D, TOK], f32, tag="pp")
            nc.tensor.transpose(lnT_ps[:], lnf[:], idm[:])
            lnT = cp.tile([D, TOK], f32, tag="lnT")
            nc.vector.tensor_copy(lnT[:], lnT_ps[:])

            xpT_ps = pprep.tile([D, TOK], f32, tag="pp")
            nc.tensor.matmul(xpT_ps[:], mt[:], lnT[:], start=True, stop=True)
            xpT = cp.tile([D, TOK], f32, tag="xpT")
            nc.vector.tensor_copy(xpT[:], xpT_ps[:])

            res_ps = pprep.tile([D, TOK], f32, tag="pp")
            nc.tensor.matmul(res_ps[:], rt[:], lnT[:], start=True, stop=True)
            res_sb = cp.tile([D, TOK], f32, tag="res_sb")
            nc.vector.tensor_copy(res_sb[:], res_ps[:])

            # res back to token-major
            resT_ps = pprep.tile([TOK, D], f32, tag="pp")
            nc.tensor.transpose(resT_ps[:], res_sb[:], id64)
            resT = cp.tile([TOK, D], f32, tag="resT")
            nc.vector.tensor_copy(resT[:], resT_ps[:])

            # xproj token-major (for the small-p pass)
            xp_ps = pprep.tile([TOK, D], f32, tag="pp")
            nc.tensor.transpose(xp_ps[:], xpT[:], id64)
            xp_sb = cp.tile([TOK, D], f32, tag="xp_sb")
            nc.vector.tensor_copy(xp_sb[:], xp_ps[:])

            # XPrep2[p, t] = xproj[t, p>>1]
            xpr_ps = pprep.tile([128, TOK], f32, tag="pp")
            nc.tensor.matmul(xpr_ps[:], rep[:], xpT[:], start=True, stop=True)
            xprep = cp.tile([128, TOK], f32, tag="xprep")
            nc.vector.tensor_copy(xprep[:], xpr_ps[:])

            # ---------------- small-p pass (token-major, i<4) ----------------
            # 1/p row gathered from invp2 (already reciprocals), then
            # broadcast to all partitions on GpSimd
            invrow = cp.tile([1, SMALL], f32, tag="invrow")
            for i in range(4):
                nc.sync.dma_start(
                    out=invrow[0:1, i * 512 : (i + 1) * 512].rearrange(
                        "a (p glo) -> a p glo", glo=4
                    ),
                    in_=invp2[:, i * 4 : i * 4 + 4],
                )
            invp0 = cp.tile([128, SMALL], f32, tag="invp0")
            p0rep = cp.tile([128, SMALL], f16, tag="p0rep")
            for ch in range(4):
                sl = slice(ch * 512, (ch + 1) * 512)
                bi_ps = pprep.tile([128, 512], f32, tag="pp")
                nc.tensor.matmul(bi_ps[:], onesr[:], invrow[:, sl], start=True, stop=True)
                nc.vector.tensor_copy(invp0[:, sl], bi_ps[:])
                bp_ps = pprep.tile([128, 512], f32, tag="pp")
                nc.tensor.matmul(bp_ps[:], onesr[:], p0r[:, sl], start=True, stop=True)
                nc.vector.tensor_copy(p0rep[:, sl], bp_ps[:])

            uu = cp.tile([128, SMALL], f32, tag="uu")
            nc.vector.tensor_scalar(uu[:], invp0[:], kvec[:], 0.25, OP.mult, OP.add)
            rr = cp.tile([128, SMALL], f32, tag="rr")
            nc.vector.tensor_scalar(rr[:], uu[:], RND_C, RND_C, OP.add, OP.subtract)
            ff = cp.tile([128, SMALL], f32, tag="ff")
            nc.vector.tensor_tensor(ff[:], uu[:], rr[:], OP.subtract)
            s0 = cp.tile([128, SMALL], f16, tag="s0")
            nc.scalar.activation(s0[:], ff[:], AF.Sin, bias=bias_z[:], scale=TWO_PI)
            prod0 = cp.tile([128, SMALL], f16, tag="prod0")
            nc.vector.tensor_tensor(prod0[:], s0[:], p0rep[:], OP.mult)
            rg = cp.tile([128, 256], f32, tag="rg")
            nc.vector.tensor_reduce(
                rg[:], prod0[:].rearrange("p (a b) -> p a b", b=NB), AX.X, OP.add
            )
            xp4 = cp.tile([128, 256], f32, tag="xp4")
            for cc in range(4):
                nc.vector.tensor_copy(xp4[:, cc * D : (cc + 1) * D], xp_sb[:])
            rgx = cp.tile([128, 256], f32, tag="rgx")
            nc.vector.tensor_tensor(rgx[:], rg[:], xp4[:], OP.mult)
            nksm = cp.tile([128, 4], f32, tag="nksm")
            nc.vector.tensor_reduce(
                nksm[:], rgx[:].rearrange("p (a b) -> p a b", b=D), AX.X, OP.add
            )

            # ---------------- main loop: groups of GRP tokens ----------------
            out_sb = cp.tile([TOK, D], f32, tag="out_sb")
            rows_sb = cp.tile([1, TOK * D], f32, tag="rows_sb")
            nb = PATH_B
            na = GRP - nb
            for g in range(NGRP):
                t0 = g * GRP
                sino = sinp.tile([128, GRP * 256], f16, tag="sino")
                if nb > 0:
                    ang = angp.tile([128, nb * 256], f32, tag="ang")
                for tau in range(GRP):
                    t = t0 + tau
                    if tau < na:
                        # path A: fused scale inside the Sin activation
                        nc.scalar.activation(
                            sino[:, tau * 256 : (tau + 1) * 256],
                            invp2g[:],
                            AF.Sin,
                            bias=bias_hp[:],
                            scale=k2pi[:, t : t + 1],
                        )
                    else:
                        # path B: angle on VectorE, sin batched below
                        b = tau - na
                        nc.vector.tensor_scalar(
                            ang[:, b * 256 : (b + 1) * 256],
                            invp2g[:],
                            k2pi[:, t : t + 1],
                            None,
                            OP.mult,
                        )
                if nb > 0:
                    nc.scalar.activation(
                        sino[:, na * 256 :],
                        ang[:],
                        AF.Sin,
                        bias=bias_hp[:],
                        scale=1.0,
                    )
                # prodx layout: (glo, tt, i) so each glo slice is contiguous
                prodx = prodp.tile([128, GRP * 256], f16, tag="prodx")
                pm4 = prodx[:].rearrange(
                    "p (glo tt i) -> p tt glo i", glo=4, tt=GRP
                )
                for tau in range(GRP):
                    t = t0 + tau
                    if USE_STT:
                        nc.vector.scalar_tensor_tensor(
                            pm4[:, tau, :, :],
                            sino[:, tau * 256 : (tau + 1) * 256].rearrange(
                                "p (glo i) -> p glo i", glo=4
                            ),
                            xprep[:, t : t + 1],
                            p2hg[:].rearrange("p (glo i) -> p glo i", glo=4),
                            OP.mult,
                            OP.mult,
                        )
                    else:
                        sx = sinp.tile([128, 256], f16, tag="sx")
                        nc.vector.tensor_scalar(
                            sx[:],
                            sino[:, tau * 256 : (tau + 1) * 256],
                            xprep[:, t : t + 1],
                            None,
                            OP.mult,
                        )
                        nc.vector.tensor_tensor(
                            pm4[:, tau, :, :],
                            sx[:].rearrange("p (glo i) -> p glo i", glo=4),
                            p2hg[:].rearrange("p (glo i) -> p glo i", glo=4),
                            OP.mult,
                        )
                # PE: partition-sum over p, accumulating the 4 glo slices
                # into this group's PSUM row
                rows = prowp.tile([1, GRP * D], f32, tag="rows")
                for glo in range(4):
                    nc.tensor.matmul(
                        rows[:],
                        ones_h[:],
                        prodx[:, glo * GRP * D : (glo + 1) * GRP * D],
                        start=(glo == 0),
                        stop=(glo == 3),
                    )
                seg = rows_sb[0:1, g * GRP * D : (g + 1) * GRP * D]
                if g % 2 == 0:
                    nc.scalar.copy(seg, rows[:])
                else:
                    nc.vector.tensor_copy(seg, rows[:])

            # ---------------- combine + output ----------------
            # row buffer is y-flat; SBUF->SBUF partition-scatter DMA is
            # broken in this stack, so bounce through an HBM scratch in
            # 4 chunks (each can start as soon as its quarter is done)
            CH = TOK * D // 4
            for c in range(4):
                nc.sync.dma_start(
                    out=scratch[c * CH : (c + 1) * CH].rearrange(
                        "(a f) -> a f", a=1
                    ),
                    in_=rows_sb[0:1, c * CH : (c + 1) * CH],
                )
                nc.sync.dma_start(
                    out=out_sb[c * (TOK // 4) : (c + 1) * (TOK // 4), :],
                    in_=scratch[c * CH : (c + 1) * CH].rearrange(
                        "(t i) -> t i", i=D
                    ),
                )
            nc.vector.tensor_tensor(out_sb[:], out_sb[:], resT[:], OP.add)
            nc.vector.tensor_tensor(
                out_sb[:, 0:4], out_sb[:, 0:4], nksm[:], OP.add
            )
            nc.sync.dma_start(out=Y[:], in_=out_sb[:])

    if split:
        _split_waits(nc)
    return nc


def kernel(x, k, M, R, P, gamma, beta, periods):
    from concourse.bass_utils import run_bass_kernel_spmd

    if "nc" not in _CACHE:
        _CACHE["nc"] = _build_program()
    nc = _CACHE["nc"]

    xf = np.ascontiguousarray(x, dtype=np.float32).reshape(B * S, D)
    kf = np.ascontiguousarray(k, dtype=np.float32).reshape(B * S)
    Mf = np.ascontiguousarray(M, dtype=np.float32)
    Rf = np.ascontiguousarray(R, dtype=np.float32)
    Pf = np.ascontiguousarray(P, dtype=np.float32).reshape(-1)
    gf = np.ascontiguousarray(gamma, dtype=np.float32).reshape(1, D)
    bf = np.ascontiguousarray(beta, dtype=np.float32).reshape(1, D)
    pf = np.ascontiguousarray(periods, dtype=np.float32).reshape(-1)

    ones = np.ones((1, 128), dtype=np.float32)
    repm = np.zeros((D, 128), dtype=np.float32)
    repm[np.arange(128) // 2, np.arange(128)] = 1.0
    idm = np.eye(128, dtype=np.float32)

    in_maps = []
    for core in range(N_CORES):
        sl = slice(core * TOK, (core + 1) * TOK)
        in_maps.append(
            {
                "x": xf[sl],
                "k": kf[sl].reshape(1, TOK),
                "M": Mf,
                "R": Rf,
                "P": Pf,
                "gamma": gf,
                "beta": bf,
                "periods": pf,
                "ones": ones,
                "rep": repm,
                "idm": idm,
            }
        )

    _CACHE["in_maps"] = in_maps
    res = run_bass_kernel_spmd(nc, in_maps, core_ids=list(range(N_CORES)))
    out = np.concatenate([res.results[c]["y"] for c in range(N_CORES)], axis=0)
    return out.reshape(B, S, D)
